# revision 1
# baseline (speedup 1.0000x reference)
"""Factored (column) attention kernel for Trainium2, 8 NeuronCores.

Reference computation (B=4, S=4096, D=1024, BLOCK_LEN=128, NB=32):
    qkv = x @ Wqkv + bqkv ; split q,k,v
    'column' attention: each (batch, within-block position bl) row attends
    causally over the NB=32 block indices -> 512 independent length-32
    single-head attentions with head dim 1024.
    out = attn @ Wout + bout

Sharding: data-parallel over the 512 independent (b, bl) attention rows,
64 rows (2048 tokens) per core.  All inputs are re-laid-out host-side so
that on-device matmuls are layout-natural:
  - x is regrouped to (group, nb, D), transposed per core, stored
    block-major so every DMA is contiguous
  - q,k are produced in transposed layout [D, tok] (lhsT = W chunk)
  - v is produced in natural layout [tok, D]  (lhsT = x^T chunk)
  - scores for a 4-group q-pack computed as one [K=128,M=128,N=256]
    matmul per d-chunk against the pack's own 8-group k half
    (cross-group products masked away in softmax)
  - softmax batched on [128,256] tiles; exp+rowsum fused via accum_out;
    normalized p transposed per 32x32 block by one DVE stream-transpose
  - p@v contracts over the 128-partition token axis; zeros in the
    block-diagonal p^T mask cross-group terms exactly
  - out projection consumes attn^T [D, tok] directly; out^T DMA'd out
Biases: bk provably cancels in softmax; bv folded host-side into
bo_eff = bout + bv @ Wout; bq/bo applied via activation bias.
Numerics: all matmul operands fp16 (fp32 PSUM accumulation); host-
simulated end-to-end rms error vs the fp32 reference is ~6e-4.
"""

import numpy as np

import concourse.bacc as bacc
import concourse.mybir as mybir
import concourse.tile as tile
from concourse.bass_utils import run_bass_kernel_spmd

N_CORES = 8
B, S, D = 4, 4096, 1024
BL = 128          # BLOCK_LEN (within-block positions)
NB = S // BL      # 32 block indices = attention sequence length
NGROUP = B * BL   # 512 independent attention rows
GPC = NGROUP // N_CORES   # 64 groups per core
TOK = GPC * NB    # 2048 tokens per core
BLK = 512         # tokens per fused block (16 groups, 4 q-packs)
NBLK = TOK // BLK  # 4
QP = BLK // 128   # q-packs per block
DC = D // 128     # 8 d-chunks
SCALE = 1.0 / np.sqrt(D)
NEG = -1.0e30

F32 = mybir.dt.float32
F16 = mybir.dt.float16

_PROGRAM = None


def _get_program():
    global _PROGRAM
    if _PROGRAM is None:
        _PROGRAM = _build_program()
    return _PROGRAM


def _build_program():
    nc = bacc.Bacc("TRN2", target_bir_lowering=False, debug=False,
                   num_devices=N_CORES)
    xt = nc.dram_tensor("xt", [NBLK * DC * 128, BLK], F16,
                        kind="ExternalInput").ap()
    wq = nc.dram_tensor("wq", [D, D], F16, kind="ExternalInput").ap()
    wk = nc.dram_tensor("wk", [D, D], F16, kind="ExternalInput").ap()
    wv = nc.dram_tensor("wv", [D, D], F16, kind="ExternalInput").ap()
    wout = nc.dram_tensor("wout", [D, D], F16, kind="ExternalInput").ap()
    bq = nc.dram_tensor("bq", [D], F32, kind="ExternalInput").ap()
    bo = nc.dram_tensor("bo", [D], F32, kind="ExternalInput").ap()
    mask = nc.dram_tensor("mask", [128, 128], F32,
                          kind="ExternalInput").ap()
    ot = nc.dram_tensor("ot", [NBLK * DC * 128, BLK], F16,
                        kind="ExternalOutput").ap()

    with tile.TileContext(nc) as tc:
        with (
            tc.tile_pool(name="wq", bufs=1) as wq_pool,
            tc.tile_pool(name="wo", bufs=1) as wo_pool,
            tc.tile_pool(name="const", bufs=1) as const,
            tc.tile_pool(name="xt", bufs=12) as xt_pool,
            tc.tile_pool(name="qk", bufs=1) as qk_pool,
            tc.tile_pool(name="v", bufs=5) as v_pool,
            tc.tile_pool(name="at", bufs=1) as at_pool,
            tc.tile_pool(name="sm", bufs=6) as sm_pool,
            tc.tile_pool(name="smh", bufs=8) as smh_pool,
            tc.tile_pool(name="small", bufs=8) as small_pool,
            tc.tile_pool(name="out", bufs=3) as out_pool,
            tc.tile_pool(name="psA", bufs=5, space="PSUM") as psA,
            tc.tile_pool(name="psB", bufs=3, space="PSUM") as psB,
        ):
            # --- staged input DMAs: consts, q-weights, first x block,
            # then k/v weights and Wout (needed progressively later).
            # warm-up matmuls on a zeroed tile: keep the PE busy (and the
            # HAM clock-gate warm) while the first weight DMAs land
            wu = const.tile([128, 512], F16, tag="warm")
            nc.vector.memset(wu[:], 0.0)
            wu_ps = psB.tile([128, 512], F32, tag="psB", name="wu_ps")
            for _ in range(40):
                nc.tensor.matmul(wu_ps[:], lhsT=wu[:, 0:128], rhs=wu[:],
                                 start=True, stop=True)
            # critical first-block loads split across HWDGE (sync) and
            # SWDGE (gpsimd) queue sets for ~2x transfer parallelism
            wq_sb = []
            for c in range(DC):
                w = wq_pool.tile([128, D], F16, tag=f"wq{c}", name=f"wq{c}")
                eng = nc.sync if c < 6 else nc.gpsimd
                eng.dma_start(w[:], wq[128 * c:128 * (c + 1), :])
                wq_sb.append(w)
            xt0_sb = []
            for c in range(DC):
                t = xt_pool.tile([128, BLK], F16, tag="xt", name="xt0")
                nc.gpsimd.dma_start(t[:], xt[128 * c:128 * (c + 1), :])
                xt0_sb.append(t)
            wk_sb = []
            for c in range(DC):
                w = wq_pool.tile([128, D], F16, tag=f"wk{c}", name=f"wk{c}")
                nc.sync.dma_start(w[:], wk[128 * c:128 * (c + 1), :])
                wk_sb.append(w)
            mask_sb = const.tile([128, 128], F32, tag="mask")
            nc.gpsimd.dma_start(mask_sb[:], mask[:])
            bq_sb = const.tile([128, DC], F32, tag="bq")
            nc.gpsimd.dma_start(bq_sb[:], bq.rearrange("(c p) -> p c", p=128))
            bo_sb = const.tile([128, DC], F32, tag="bo")
            nc.gpsimd.dma_start(bo_sb[:], bo.rearrange("(c p) -> p c", p=128))
            wv_sb = []
            for c in range(DC):
                w = wq_pool.tile([128, D], F16, tag=f"wv{c}", name=f"wv{c}")
                nc.sync.dma_start(w[:], wv[128 * c:128 * (c + 1), :])
                wv_sb.append(w)
            wo_sb = []
            for c in range(DC):
                w2 = wo_pool.tile([128, D], F16, tag=f"wo{c}", name=f"wo{c}")
                nc.sync.dma_start(w2[:], wout[128 * c:128 * (c + 1), :])
                wo_sb.append(w2)

            for b in range(NBLK):
                if b == 0:
                    xt_sb = xt0_sb
                else:
                    xt_sb = []
                    for c in range(DC):
                        r0 = (b * DC + c) * 128
                        t = xt_pool.tile([128, BLK], F16, tag="xt", name="xt")
                        nc.sync.dma_start(t[:], xt[r0:r0 + 128, :])
                        xt_sb.append(t)

                # --- q^T, k^T projections: psum [dout-chunk 128, BLK tok]
                # interleaved so scores' (q_c, k_c) pairs are ready early
                qk_sb = [None] * (2 * DC)
                if b == 0:
                    jorder = list(range(2 * DC))
                else:
                    jorder = [x for p_ in zip(range(DC), range(DC, 2 * DC))
                              for x in p_]
                for j in jorder:
                    wlist = wq_sb if j < DC else wk_sb
                    jj = j % DC
                    ps = psA.tile([128, BLK], F32, tag="psA")
                    for c in range(DC):
                        nc.tensor.matmul(
                            ps[:],
                            lhsT=wlist[c][:, 128 * jj:128 * (jj + 1)],
                            rhs=xt_sb[c][:],
                            start=(c == 0), stop=(c == DC - 1),
                        )
                    q = qk_pool.tile([128, BLK], F16, tag=f"qk{j}")
                    if j < DC:
                        nc.scalar.add(q[:], ps[:], bq_sb[:, j:j + 1])
                    else:
                        nc.scalar.copy(q[:], ps[:])
                    qk_sb[j] = q

                # --- scores + softmax per 4-group q-pack (before the v
                # projection so the softmax chain hides behind v matmuls)
                pt_sb = []
                for qp in range(QP):
                    ps = psB.tile([128, 128], F32, tag="psB")
                    for c in range(DC):
                        nc.tensor.matmul(
                            ps[:],
                            lhsT=qk_sb[c][:, 128 * qp:128 * (qp + 1)],
                            rhs=qk_sb[DC + c][:, 128 * qp:128 * (qp + 1)],
                            start=(c == 0), stop=(c == DC - 1),
                        )
                    tm = sm_pool.tile([128, 128], F32, tag="sm")
                    nc.vector.tensor_add(tm[:], ps[:], mask_sb[:])
                    p4 = sm_pool.tile([128, 128], F32, tag="sm")
                    s4 = small_pool.tile([128, 1], F32, tag="s4")
                    nc.scalar.activation(
                        p4[:], tm[:], mybir.ActivationFunctionType.Exp,
                        scale=float(SCALE), accum_out=s4[:],
                    )
                    r4 = small_pool.tile([128, 1], F32, tag="r4")
                    nc.vector.reciprocal(r4[:], s4[:])
                    pn = smh_pool.tile([128, 128], F16, tag="smh")
                    nc.vector.tensor_scalar_mul(pn[:], p4[:], r4[:])
                    pt = smh_pool.tile([128, 128], F16, tag="smh")
                    nc.vector.transpose(pt[:], pn[:])
                    pt_sb.append(pt)

                # --- v natural: psum [tok-chunk 128, 512 dout]
                v_sb = []
                for tch in range(QP):
                    vt = v_pool.tile([128, D], F16, tag="v")
                    for hh in range(2):
                        ps = psA.tile([128, 512], F32, tag="psA")
                        for c in range(DC):
                            nc.tensor.matmul(
                                ps[:],
                                lhsT=xt_sb[c][:, 128 * tch:128 * (tch + 1)],
                                rhs=wv_sb[c][:, 512 * hh:512 * (hh + 1)],
                                start=(c == 0), stop=(c == DC - 1),
                            )
                        nc.vector.tensor_copy(vt[:, 512 * hh:512 * (hh + 1)],
                                              ps[:])
                    v_sb.append(vt)

                # --- p @ v  -> attn^T [d-chunk 128, BLK tok]
                at_sb = [at_pool.tile([128, BLK], F16, tag=f"at{c}",
                                      name=f"at{c}")
                         for c in range(DC)]
                for qpair in range(QP // 2):
                    qp0, qp1 = 2 * qpair, 2 * qpair + 1
                    for c in range(DC):
                        ps = psB.tile([128, 256], F32, tag="psB")
                        nc.tensor.matmul(
                            ps[:, 0:128],
                            lhsT=v_sb[qp0][:, 128 * c:128 * (c + 1)],
                            rhs=pt_sb[qp0][:],
                            start=True, stop=True,
                        )
                        nc.tensor.matmul(
                            ps[:, 128:256],
                            lhsT=v_sb[qp1][:, 128 * c:128 * (c + 1)],
                            rhs=pt_sb[qp1][:],
                            start=True, stop=True, skip_group_check=True,
                        )
                        dst = at_sb[c][:, 256 * qpair:256 * (qpair + 1)]
                        if c % 2 == 0:
                            nc.vector.tensor_copy(dst, ps[:])
                        else:
                            nc.scalar.copy(dst, ps[:])

                # --- out projection: psum [dout-chunk 128, BLK tok]
                for j in range(DC):
                    ps = psA.tile([128, BLK], F32, tag="psA")
                    for c in range(DC):
                        nc.tensor.matmul(
                            ps[:],
                            lhsT=wo_sb[c][:, 128 * j:128 * (j + 1)],
                            rhs=at_sb[c][:],
                            start=(c == 0), stop=(c == DC - 1),
                        )
                    o = out_pool.tile([128, BLK], F16, tag="o")
                    r0 = (b * DC + j) * 128
                    if b < NBLK - 1:
                        if j % 2 == 0:
                            nc.scalar.add(o[:], ps[:], bo_sb[:, j:j + 1])
                        else:
                            nc.vector.tensor_scalar_add(o[:], ps[:],
                                                        bo_sb[:, j:j + 1])
                        nc.sync.dma_start(ot[r0:r0 + 128, :], o[:])
                    else:
                        # final block: half-width evict+store on two engines/
                        # queues so the kernel tail drains twice as fast
                        nc.scalar.add(o[:, 0:BLK // 2], ps[:, 0:BLK // 2],
                                      bo_sb[:, j:j + 1])
                        nc.vector.tensor_scalar_add(
                            o[:, BLK // 2:BLK], ps[:, BLK // 2:BLK],
                            bo_sb[:, j:j + 1])
                        nc.sync.dma_start(ot[r0:r0 + 128, 0:BLK // 2],
                                          o[:, 0:BLK // 2])
                        nc.sync.dma_start(ot[r0:r0 + 128, BLK // 2:BLK],
                                          o[:, BLK // 2:BLK])

    nc.compile()
    return nc


def _make_mask():
    """One [128, 128] additive-mask tile shared by every q-pack: rows
    and columns are the pack's own 4 groups x 32 positions; the group-
    diagonal blocks carry the causal mask, everything else NEG
    (-> exp == 0 exactly)."""
    m = np.full((128, 128), NEG, dtype=np.float32)
    for i in range(4):
        for q in range(NB):
            m[32 * i + q, 32 * i:32 * i + q + 1] = 0.0
    return m


def run(x, Wqkv, bqkv, Wout, bout, trace=False):
    x = np.asarray(x, dtype=np.float32)
    Wqkv = np.asarray(Wqkv, dtype=np.float32)
    bqkv = np.asarray(bqkv, dtype=np.float32)
    Wout = np.asarray(Wout, dtype=np.float32)
    bout = np.asarray(bout, dtype=np.float32)

    # (B, S, D) -> (group, nb, D), group = b*BL + bl, token = g*NB + nb
    xg = x.reshape(B, NB, BL, D).transpose(0, 2, 1, 3).reshape(NGROUP, NB, D)
    bq = np.ascontiguousarray(bqkv[:D])
    bv = bqkv[2 * D:3 * D]
    bo = np.ascontiguousarray(bout + bv @ Wout)
    mask = _make_mask()
    wq = np.ascontiguousarray(Wqkv[:, :D]).astype(np.float16)
    wk = np.ascontiguousarray(Wqkv[:, D:2 * D]).astype(np.float16)
    wv = np.ascontiguousarray(Wqkv[:, 2 * D:3 * D]).astype(np.float16)
    wo = Wout.astype(np.float16)

    nc = _get_program()
    in_maps = []
    for i in range(N_CORES):
        xt_i = xg[GPC * i:GPC * (i + 1)].reshape(TOK, D).T
        # block-major layout: [NBLK, DC, 128, BLK] rows contiguous
        xt_i = np.ascontiguousarray(
            xt_i.reshape(DC, 128, NBLK, BLK).transpose(2, 0, 1, 3)
            .reshape(NBLK * DC * 128, BLK)).astype(np.float16)
        in_maps.append({
            "xt": xt_i, "wq": wq, "wk": wk, "wv": wv, "wout": wo,
            "bq": bq, "bo": bo, "mask": mask,
        })
    res = run_bass_kernel_spmd(nc, in_maps, list(range(N_CORES)), trace=trace)

    outs = np.empty((NGROUP, NB, D), dtype=np.float32)
    for i in range(N_CORES):
        ot_i = (res.results[i]["ot"].astype(np.float32)
                .reshape(NBLK, DC, 128, BLK)
                .transpose(1, 2, 0, 3).reshape(D, TOK))
        outs[GPC * i:GPC * (i + 1)] = ot_i.T.reshape(GPC, NB, D)
    out = (outs.reshape(B, BL, NB, D).transpose(0, 2, 1, 3)
           .reshape(B, S, D))
    return out, res


def kernel(x, Wqkv, bqkv, Wout, bout):
    out, _ = run(x, Wqkv, bqkv, Wout, bout, trace=False)
    return out



# revision 2
# speedup vs baseline: 2.0485x; 2.0485x over previous
"""Factored (column) attention kernel for Trainium2, 8 NeuronCores.

Reference computation (B=4, S=4096, D=1024, BLOCK_LEN=128, NB=32):
    qkv = x @ Wqkv + bqkv ; split q,k,v
    'column' attention: each (batch, within-block position bl) row attends
    causally over the NB=32 block indices -> 512 independent length-32
    single-head attentions with head dim 1024.
    out = attn @ Wout + bout

Algebraic fold (halves device matmul work vs the 4-GEMM formulation):
  scores = (x Wq + bq) . (x Wk)  =  (x M + bq_eff) . x,  M = Wq Wk^T,
           bq_eff = Wk bq           (bk cancels in softmax)
  out    = p @ (x Wv + bv) Wout + bout = p @ (x N) + bo_eff,
           N = Wv Wout, bo_eff = bout + bv Wout   (rows of p sum to 1)
so the device only computes TWO [tok,1024]x[1024,1024] GEMMs (q'' = x M,
vout = x N) plus the tiny 32-long attentions; k/v/out projections vanish.
M, N, bq_eff, bo_eff are precomputed host-side in fp32.

Sharding: data-parallel over the 512 independent (b, bl) attention rows,
64 rows (2048 tokens) per core.  x is staged host-side as x^T, block-major
so every DMA is contiguous:
  - q'' produced in transposed layout [D, tok] (lhsT = M chunk)
  - scores for a 4-group q-pack: one [K=128,M=128,N=128] matmul per
    d-chunk of q''^T against the SAME x^T chunk (no k projection)
  - softmax on [128,128] tiles; exp+rowsum fused via accum_out;
    normalized p transposed by one DVE stream-transpose
  - vout = x N in natural layout [tok, D] (lhsT = x^T chunk)
  - out = p @ vout via lhsT = p^T: psum [128 q-tok, 512 d] natural, so
    the output DMA is 2KB-contiguous rows; bo_eff added host-side
Numerics: all matmul operands fp16 (fp32 PSUM accumulation); host-
simulated end-to-end rms error vs the fp32 reference is ~5.3e-4.
"""

import numpy as np

import concourse.bacc as bacc
import concourse.mybir as mybir
import concourse.tile as tile
from concourse.bass_utils import run_bass_kernel_spmd

N_CORES = 8
B, S, D = 4, 4096, 1024
BL = 128          # BLOCK_LEN (within-block positions)
NB = S // BL      # 32 block indices = attention sequence length
NGROUP = B * BL   # 512 independent attention rows
GPC = NGROUP // N_CORES   # 64 groups per core
TOK = GPC * NB    # 2048 tokens per core
BLK = 512         # tokens per fused block (16 groups, 4 q-packs)
NBLK = TOK // BLK  # 4
QP = BLK // 128   # q-packs per block
DC = D // 128     # 8 d-chunks
SCALE = 1.0 / np.sqrt(D)
NEG = -1.0e30
WARMUP = 20       # PE warm-up matmuls covering the first weight DMAs

F32 = mybir.dt.float32
F16 = mybir.dt.float16

_PROGRAM = None


def _get_program():
    global _PROGRAM
    if _PROGRAM is None:
        _PROGRAM = _build_program()
    return _PROGRAM


def _build_program():
    nc = bacc.Bacc("TRN2", target_bir_lowering=False, debug=False,
                   num_devices=N_CORES)
    xt = nc.dram_tensor("xt", [NBLK * DC * 128, BLK], F16,
                        kind="ExternalInput").ap()
    wm = nc.dram_tensor("wm", [D, D], F16, kind="ExternalInput").ap()
    wn = nc.dram_tensor("wn", [D, D], F16, kind="ExternalInput").ap()
    bq = nc.dram_tensor("bq", [D], F32, kind="ExternalInput").ap()
    mask = nc.dram_tensor("mask", [128, 128], F32,
                          kind="ExternalInput").ap()
    ot = nc.dram_tensor("ot", [TOK, D], F16, kind="ExternalOutput").ap()

    with tile.TileContext(nc) as tc:
        with (
            tc.tile_pool(name="w", bufs=1) as w_pool,
            tc.tile_pool(name="const", bufs=1) as const,
            tc.tile_pool(name="xt", bufs=32) as xt_pool,
            tc.tile_pool(name="q", bufs=2) as q_pool,
            tc.tile_pool(name="v", bufs=5) as v_pool,
            tc.tile_pool(name="sm", bufs=6) as sm_pool,
            tc.tile_pool(name="smh", bufs=8) as smh_pool,
            tc.tile_pool(name="small", bufs=8) as small_pool,
            tc.tile_pool(name="out", bufs=3) as out_pool,
            tc.tile_pool(name="psA", bufs=5, space="PSUM") as psA,
            tc.tile_pool(name="psB", bufs=3, space="PSUM") as psB,
        ):
            # --- staged input DMAs.  warm-up matmuls on a zeroed tile
            # keep the PE busy (and the HAM clock-gate warm) while the
            # first weight DMAs land
            wu = const.tile([128, 512], F16, tag="warm")
            nc.vector.memset(wu[:], 0.0)
            wu_ps = psB.tile([128, 512], F32, tag="psB", name="wu_ps")
            for _ in range(WARMUP):
                nc.tensor.matmul(wu_ps[:], lhsT=wu[:, 0:128], rhs=wu[:],
                                 start=True, stop=True)
            # critical first-block loads split across HWDGE (sync) and
            # SWDGE (gpsimd) queue sets for ~2x transfer parallelism
            wm_sb = []
            for c in range(DC):
                w = w_pool.tile([128, D], F16, tag=f"wm{c}", name=f"wm{c}")
                eng = nc.sync if c < 6 else nc.gpsimd
                eng.dma_start(w[:], wm[128 * c:128 * (c + 1), :])
                wm_sb.append(w)
            xt0_sb = []
            for c in range(DC):
                t = xt_pool.tile([128, BLK], F16, tag="xt", name="xt0")
                nc.gpsimd.dma_start(t[:], xt[128 * c:128 * (c + 1), :])
                xt0_sb.append(t)
            mask_sb = const.tile([128, 128], F32, tag="mask")
            nc.gpsimd.dma_start(mask_sb[:], mask[:])
            bq_sb = const.tile([128, DC], F32, tag="bq")
            nc.gpsimd.dma_start(bq_sb[:], bq.rearrange("(c p) -> p c", p=128))
            wn_sb = []
            for c in range(DC):
                w = w_pool.tile([128, D], F16, tag=f"wn{c}", name=f"wn{c}")
                nc.sync.dma_start(w[:], wn[128 * c:128 * (c + 1), :])
                wn_sb.append(w)
            # prefetch x^T for block 1 (block b issues block b+2's)
            xt_blocks = {0: xt0_sb}
            def _prefetch(b):
                lst = []
                for c in range(DC):
                    r0 = (b * DC + c) * 128
                    t = xt_pool.tile([128, BLK], F16, tag="xt", name="xt")
                    nc.sync.dma_start(t[:], xt[r0:r0 + 128, :])
                    lst.append(t)
                xt_blocks[b] = lst
            _prefetch(1)

            for b in range(NBLK):
                if b + 2 < NBLK:
                    _prefetch(b + 2)
                xt_sb = xt_blocks.pop(b)

                # --- q''^T projection: psum [dout-chunk 128, BLK tok]
                q_sb = []
                for j in range(DC):
                    ps = psA.tile([128, BLK], F32, tag="psA")
                    for c in range(DC):
                        nc.tensor.matmul(
                            ps[:],
                            lhsT=wm_sb[c][:, 128 * j:128 * (j + 1)],
                            rhs=xt_sb[c][:],
                            start=(c == 0), stop=(c == DC - 1),
                        )
                    q = q_pool.tile([128, BLK], F16, tag=f"q{j}",
                                    name=f"q{j}")
                    nc.scalar.add(q[:], ps[:], bq_sb[:, j:j + 1])
                    q_sb.append(q)

                # --- scores + softmax per 4-group q-pack (before the
                # vout projection so the softmax chain hides behind it)
                pt_sb = []
                for qp in range(QP):
                    ps = psB.tile([128, 128], F32, tag="psB")
                    for c in range(DC):
                        nc.tensor.matmul(
                            ps[:],
                            lhsT=q_sb[c][:, 128 * qp:128 * (qp + 1)],
                            rhs=xt_sb[c][:, 128 * qp:128 * (qp + 1)],
                            start=(c == 0), stop=(c == DC - 1),
                        )
                    tm = sm_pool.tile([128, 128], F32, tag="sm")
                    nc.vector.tensor_add(tm[:], ps[:], mask_sb[:])
                    p4 = sm_pool.tile([128, 128], F32, tag="sm")
                    s4 = small_pool.tile([128, 1], F32, tag="s4")
                    nc.scalar.activation(
                        p4[:], tm[:], mybir.ActivationFunctionType.Exp,
                        scale=float(SCALE), accum_out=s4[:],
                    )
                    r4 = small_pool.tile([128, 1], F32, tag="r4")
                    nc.vector.reciprocal(r4[:], s4[:])
                    pn = smh_pool.tile([128, 128], F16, tag="smh")
                    nc.vector.tensor_scalar_mul(pn[:], p4[:], r4[:])
                    pt = smh_pool.tile([128, 128], F16, tag="smh")
                    nc.vector.transpose(pt[:], pn[:])
                    pt_sb.append(pt)

                # --- vout = x N natural: psum [tok-chunk 128, 512 dout]
                v_sb = []
                for tch in range(QP):
                    vt = v_pool.tile([128, D], F16, tag="v")
                    for hh in range(2):
                        ps = psA.tile([128, 512], F32, tag="psA")
                        for c in range(DC):
                            nc.tensor.matmul(
                                ps[:],
                                lhsT=xt_sb[c][:, 128 * tch:128 * (tch + 1)],
                                rhs=wn_sb[c][:, 512 * hh:512 * (hh + 1)],
                                start=(c == 0), stop=(c == DC - 1),
                            )
                        if hh == 0:
                            nc.vector.tensor_copy(
                                vt[:, 512 * hh:512 * (hh + 1)], ps[:])
                        else:
                            nc.scalar.copy(
                                vt[:, 512 * hh:512 * (hh + 1)], ps[:])
                    v_sb.append(vt)

                # --- out = p @ vout: psum [128 q-tok, 512 d] natural,
                # evict split across scalar/vector, 2KB-row DMA out
                for qp in range(QP):
                    o = out_pool.tile([128, D], F16, tag="o")
                    for hh in range(2):
                        ps = psB.tile([128, 512], F32, tag="psB")
                        nc.tensor.matmul(
                            ps[:],
                            lhsT=pt_sb[qp][:],
                            rhs=v_sb[qp][:, 512 * hh:512 * (hh + 1)],
                            start=True, stop=True,
                        )
                        if hh == 0:
                            nc.scalar.copy(o[:, 0:512], ps[:])
                        else:
                            nc.vector.tensor_copy(o[:, 512:1024], ps[:])
                    r0 = (b * QP + qp) * 128
                    nc.gpsimd.dma_start(ot[r0:r0 + 128, :], o[:])

    nc.compile()
    return nc


def _make_mask():
    """One [128, 128] additive-mask tile shared by every q-pack: rows
    and columns are the pack's own 4 groups x 32 positions; the group-
    diagonal blocks carry the causal mask, everything else NEG
    (-> exp == 0 exactly)."""
    m = np.full((128, 128), NEG, dtype=np.float32)
    for i in range(4):
        for q in range(NB):
            m[32 * i + q, 32 * i:32 * i + q + 1] = 0.0
    return m


def run(x, Wqkv, bqkv, Wout, bout, trace=False):
    x = np.asarray(x, dtype=np.float32)
    Wqkv = np.asarray(Wqkv, dtype=np.float32)
    bqkv = np.asarray(bqkv, dtype=np.float32)
    Wout = np.asarray(Wout, dtype=np.float32)
    bout = np.asarray(bout, dtype=np.float32)

    Wq, Wk, Wv = Wqkv[:, :D], Wqkv[:, D:2 * D], Wqkv[:, 2 * D:]
    wm = (Wq @ Wk.T).astype(np.float16)
    wn = (Wv @ Wout).astype(np.float16)
    bq_eff = np.ascontiguousarray(Wk @ bqkv[:D])
    bo_eff = (bout + bqkv[2 * D:] @ Wout).astype(np.float32)
    mask = _make_mask()

    # (B, S, D) -> (group, nb, D), group = b*BL + bl, token = g*NB + nb
    xg = x.reshape(B, NB, BL, D).transpose(0, 2, 1, 3).reshape(NGROUP, NB, D)

    nc = _get_program()
    in_maps = []
    for i in range(N_CORES):
        xt_i = xg[GPC * i:GPC * (i + 1)].reshape(TOK, D).T
        # block-major layout: [NBLK, DC, 128, BLK] rows contiguous
        xt_i = np.ascontiguousarray(
            xt_i.reshape(DC, 128, NBLK, BLK).transpose(2, 0, 1, 3)
            .reshape(NBLK * DC * 128, BLK)).astype(np.float16)
        in_maps.append({
            "xt": xt_i, "wm": wm, "wn": wn, "bq": bq_eff, "mask": mask,
        })
    res = run_bass_kernel_spmd(nc, in_maps, list(range(N_CORES)), trace=trace)

    outs = np.empty((NGROUP, NB, D), dtype=np.float32)
    for i in range(N_CORES):
        outs[GPC * i:GPC * (i + 1)] = (
            res.results[i]["ot"].astype(np.float32).reshape(GPC, NB, D))
    out = (outs.reshape(B, BL, NB, D).transpose(0, 2, 1, 3)
           .reshape(B, S, D)) + bo_eff
    return out, res


def kernel(x, Wqkv, bqkv, Wout, bout):
    out, _ = run(x, Wqkv, bqkv, Wout, bout, trace=False)
    return out


# revision 7
# speedup vs baseline: 2.0499x; 1.0007x over previous
"""Factored (column) attention kernel for Trainium2, 8 NeuronCores.

Reference computation (B=4, S=4096, D=1024, BLOCK_LEN=128, NB=32):
    qkv = x @ Wqkv + bqkv ; split q,k,v
    'column' attention: each (batch, within-block position bl) row attends
    causally over the NB=32 block indices -> 512 independent length-32
    single-head attentions with head dim 1024.
    out = attn @ Wout + bout

Algebraic fold (halves device matmul work vs the 4-GEMM formulation):
  scores = (x Wq + bq) . (x Wk)  =  (x M + bq_eff) . x,  M = Wq Wk^T,
           bq_eff = Wk bq           (bk cancels in softmax)
  out    = p @ (x Wv + bv) Wout + bout = p @ (x N) + bo_eff,
           N = Wv Wout, bo_eff = bout + bv Wout   (rows of p sum to 1)
so the device only computes TWO [tok,1024]x[1024,1024] GEMMs (q'' = x M,
vout = x N) plus the tiny 32-long attentions; k/v/out projections vanish.
M, N, bq_eff, bo_eff are precomputed host-side in fp32.

Sharding: data-parallel over the 512 independent (b, bl) attention rows,
64 rows (2048 tokens) per core.  x is staged host-side as x^T, block-major
so every DMA is contiguous:
  - q'' produced in transposed layout [D, tok] (lhsT = M chunk)
  - scores for a 4-group q-pack: one [K=128,M=128,N=128] matmul per
    d-chunk of q''^T against the SAME x^T chunk (no k projection)
  - softmax on [128,128] tiles; exp+rowsum fused via accum_out;
    normalized p transposed by one DVE stream-transpose
  - vout = x N in natural layout [tok, D] (lhsT = x^T chunk)
  - out = p @ vout via lhsT = p^T: psum [128 q-tok, 512 d] natural, so
    the output DMA is 2KB-contiguous rows; bo_eff added host-side
Numerics: all matmul operands fp16 (fp32 PSUM accumulation); host-
simulated end-to-end rms error vs the fp32 reference is ~5.3e-4.
"""

import numpy as np

import concourse.bacc as bacc
import concourse.mybir as mybir
import concourse.tile as tile
from concourse.bass_utils import run_bass_kernel_spmd

N_CORES = 8
B, S, D = 4, 4096, 1024
BL = 128          # BLOCK_LEN (within-block positions)
NB = S // BL      # 32 block indices = attention sequence length
NGROUP = B * BL   # 512 independent attention rows
GPC = NGROUP // N_CORES   # 64 groups per core
TOK = GPC * NB    # 2048 tokens per core
BLK = 512         # tokens per fused block (16 groups, 4 q-packs)
NBLK = TOK // BLK  # 4
QP = BLK // 128   # q-packs per block
DC = D // 128     # 8 d-chunks
SCALE = 1.0 / np.sqrt(D)
NEG = -1.0e30
WARMUP = 16       # PE warm-up matmuls covering the first weight DMAs

F32 = mybir.dt.float32
F16 = mybir.dt.float16

_PROGRAM = None


def _get_program():
    global _PROGRAM
    if _PROGRAM is None:
        _PROGRAM = _build_program()
    return _PROGRAM


def _build_program():
    nc = bacc.Bacc("TRN2", target_bir_lowering=False, debug=False,
                   num_devices=N_CORES)
    xt = nc.dram_tensor("xt", [NBLK * DC * 128, BLK], F16,
                        kind="ExternalInput").ap()
    wm = nc.dram_tensor("wm", [D, D], F16, kind="ExternalInput").ap()
    wn = nc.dram_tensor("wn", [D, D], F16, kind="ExternalInput").ap()
    bq = nc.dram_tensor("bq", [D], F32, kind="ExternalInput").ap()
    mask = nc.dram_tensor("mask", [128, 128], F32,
                          kind="ExternalInput").ap()
    ot = nc.dram_tensor("ot", [TOK, D], F16, kind="ExternalOutput").ap()

    with tile.TileContext(nc) as tc:
        with (
            tc.tile_pool(name="w", bufs=1) as w_pool,
            tc.tile_pool(name="const", bufs=1) as const,
            tc.tile_pool(name="xt", bufs=32) as xt_pool,
            tc.tile_pool(name="q", bufs=2) as q_pool,
            tc.tile_pool(name="v", bufs=5) as v_pool,
            tc.tile_pool(name="sm", bufs=6) as sm_pool,
            tc.tile_pool(name="smh", bufs=8) as smh_pool,
            tc.tile_pool(name="small", bufs=8) as small_pool,
            tc.tile_pool(name="out", bufs=3) as out_pool,
            tc.tile_pool(name="psA", bufs=5, space="PSUM") as psA,
            tc.tile_pool(name="psB", bufs=3, space="PSUM") as psB,
        ):
            # --- staged input DMAs.  warm-up matmuls on a zeroed tile
            # keep the PE busy (and the HAM clock-gate warm) while the
            # first weight DMAs land
            wu = const.tile([128, 512], F16, tag="warm")
            nc.vector.memset(wu[:], 0.0)
            wu_ps = psB.tile([128, 512], F32, tag="psB", name="wu_ps")
            for _ in range(WARMUP):
                nc.tensor.matmul(wu_ps[:], lhsT=wu[:, 0:128], rhs=wu[:],
                                 start=True, stop=True)
            # critical first-block loads spread over the three DMA-capable
            # engine queue sets (scalar is idle until the first evicts) so
            # the PE's q'' supply lands sooner than sync+gpsimd alone
            engs = [nc.sync, nc.gpsimd, nc.scalar]
            wm_sb = []
            for c in range(DC):
                w = w_pool.tile([128, D], F16, tag=f"wm{c}", name=f"wm{c}")
                engs[c % 3].dma_start(w[:], wm[128 * c:128 * (c + 1), :])
                wm_sb.append(w)
            xt0_sb = []
            for c in range(DC):
                t = xt_pool.tile([128, BLK], F16, tag="xt", name="xt0")
                engs[(c + 1) % 3].dma_start(t[:], xt[128 * c:128 * (c + 1), :])
                xt0_sb.append(t)
            mask_sb = const.tile([128, 128], F32, tag="mask")
            nc.gpsimd.dma_start(mask_sb[:], mask[:])
            bq_sb = const.tile([128, DC], F32, tag="bq")
            nc.gpsimd.dma_start(bq_sb[:], bq.rearrange("(c p) -> p c", p=128))
            wn_sb = []
            for c in range(DC):
                w = w_pool.tile([128, D], F16, tag=f"wn{c}", name=f"wn{c}")
                eng = nc.sync if c % 2 == 0 else nc.gpsimd
                eng.dma_start(w[:], wn[128 * c:128 * (c + 1), :])
                wn_sb.append(w)
            # prefetch x^T for block 1 (block b issues block b+2's)
            xt_blocks = {0: xt0_sb}
            def _prefetch(b):
                lst = []
                for c in range(DC):
                    r0 = (b * DC + c) * 128
                    t = xt_pool.tile([128, BLK], F16, tag="xt", name="xt")
                    nc.sync.dma_start(t[:], xt[r0:r0 + 128, :])
                    lst.append(t)
                xt_blocks[b] = lst
            _prefetch(1)

            for b in range(NBLK):
                if b + 2 < NBLK:
                    _prefetch(b + 2)
                xt_sb = xt_blocks.pop(b)

                # --- q''^T projection: psum [dout-chunk 128, BLK tok]
                q_sb = []
                for j in range(DC):
                    ps = psA.tile([128, BLK], F32, tag="psA")
                    for c in range(DC):
                        nc.tensor.matmul(
                            ps[:],
                            lhsT=wm_sb[c][:, 128 * j:128 * (j + 1)],
                            rhs=xt_sb[c][:],
                            start=(c == 0), stop=(c == DC - 1),
                        )
                    q = q_pool.tile([128, BLK], F16, tag=f"q{j}",
                                    name=f"q{j}")
                    nc.scalar.add(q[:], ps[:], bq_sb[:, j:j + 1])
                    q_sb.append(q)

                # --- scores + softmax per 4-group q-pack (before the
                # vout projection so the softmax chain hides behind it)
                pt_sb = []
                for qp in range(QP):
                    ps = psB.tile([128, 128], F32, tag="psB")
                    for c in range(DC):
                        nc.tensor.matmul(
                            ps[:],
                            lhsT=q_sb[c][:, 128 * qp:128 * (qp + 1)],
                            rhs=xt_sb[c][:, 128 * qp:128 * (qp + 1)],
                            start=(c == 0), stop=(c == DC - 1),
                        )
                    tm = sm_pool.tile([128, 128], F32, tag="sm")
                    nc.vector.tensor_add(tm[:], ps[:], mask_sb[:])
                    p4 = sm_pool.tile([128, 128], F32, tag="sm")
                    s4 = small_pool.tile([128, 1], F32, tag="s4")
                    nc.scalar.activation(
                        p4[:], tm[:], mybir.ActivationFunctionType.Exp,
                        scale=float(SCALE), accum_out=s4[:],
                    )
                    r4 = small_pool.tile([128, 1], F32, tag="r4")
                    nc.vector.reciprocal(r4[:], s4[:])
                    pn = smh_pool.tile([128, 128], F16, tag="smh")
                    nc.vector.tensor_scalar_mul(pn[:], p4[:], r4[:])
                    pt = smh_pool.tile([128, 128], F16, tag="smh")
                    nc.vector.transpose(pt[:], pn[:])
                    pt_sb.append(pt)

                # --- vout = x N natural: psum [tok-chunk 128, 512 dout],
                # then out = p @ vout: psum [128 q-tok, 512 d] natural,
                # each half evicted + DMA'd out as soon as it's ready.
                # Last block: pv(qp) trails vout(tch) by one stage so the
                # tail output DMAs start draining earlier, spread across
                # all four engine queues.
                def _vout(tch):
                    vt = v_pool.tile([128, D], F16, tag="v", name="vt")
                    for hh in range(2):
                        ps = psA.tile([128, 512], F32, tag="psA")
                        for c in range(DC):
                            nc.tensor.matmul(
                                ps[:],
                                lhsT=xt_sb[c][:, 128 * tch:128 * (tch + 1)],
                                rhs=wn_sb[c][:, 512 * hh:512 * (hh + 1)],
                                start=(c == 0), stop=(c == DC - 1),
                            )
                        if hh == 0:
                            nc.vector.tensor_copy(
                                vt[:, 512 * hh:512 * (hh + 1)], ps[:])
                        else:
                            nc.scalar.copy(
                                vt[:, 512 * hh:512 * (hh + 1)], ps[:])
                    return vt

                def _pv(qp, vt):
                    o = out_pool.tile([128, D], F16, tag="o", name="o")
                    r0 = (b * QP + qp) * 128
                    if b == NBLK - 1:
                        dmae = [(nc.sync, nc.gpsimd), (nc.scalar, nc.sync),
                                (nc.gpsimd, nc.scalar), (nc.sync, nc.gpsimd)
                                ][qp]
                    else:
                        dmae = (nc.sync, nc.gpsimd)
                    for hh in range(2):
                        ps = psB.tile([128, 512], F32, tag="psB")
                        nc.tensor.matmul(
                            ps[:],
                            lhsT=pt_sb[qp][:],
                            rhs=vt[:, 512 * hh:512 * (hh + 1)],
                            start=True, stop=True,
                        )
                        sl = slice(512 * hh, 512 * (hh + 1))
                        if hh == 0:
                            nc.scalar.copy(o[:, sl], ps[:])
                        else:
                            nc.vector.tensor_copy(o[:, sl], ps[:])
                        dmae[hh].dma_start(ot[r0:r0 + 128, sl], o[:, sl])

                if b < NBLK - 1:
                    v_sb = [_vout(tch) for tch in range(QP)]
                    for qp in range(QP):
                        _pv(qp, v_sb[qp])
                else:
                    v_sb = [_vout(0), _vout(1)]
                    _pv(0, v_sb[0])
                    v_sb.append(_vout(2))
                    _pv(1, v_sb[1])
                    v_sb.append(_vout(3))
                    _pv(2, v_sb[2])
                    _pv(3, v_sb[3])

    nc.compile()
    return nc


def _make_mask():
    """One [128, 128] additive-mask tile shared by every q-pack: rows
    and columns are the pack's own 4 groups x 32 positions; the group-
    diagonal blocks carry the causal mask, everything else NEG
    (-> exp == 0 exactly)."""
    m = np.full((128, 128), NEG, dtype=np.float32)
    for i in range(4):
        for q in range(NB):
            m[32 * i + q, 32 * i:32 * i + q + 1] = 0.0
    return m


def run(x, Wqkv, bqkv, Wout, bout, trace=False):
    x = np.asarray(x, dtype=np.float32)
    Wqkv = np.asarray(Wqkv, dtype=np.float32)
    bqkv = np.asarray(bqkv, dtype=np.float32)
    Wout = np.asarray(Wout, dtype=np.float32)
    bout = np.asarray(bout, dtype=np.float32)

    Wq, Wk, Wv = Wqkv[:, :D], Wqkv[:, D:2 * D], Wqkv[:, 2 * D:]
    wm = (Wq @ Wk.T).astype(np.float16)
    wn = (Wv @ Wout).astype(np.float16)
    bq_eff = np.ascontiguousarray(Wk @ bqkv[:D])
    bo_eff = (bout + bqkv[2 * D:] @ Wout).astype(np.float32)
    mask = _make_mask()

    # (B, S, D) -> (group, nb, D), group = b*BL + bl, token = g*NB + nb
    xg = x.reshape(B, NB, BL, D).transpose(0, 2, 1, 3).reshape(NGROUP, NB, D)

    nc = _get_program()
    in_maps = []
    for i in range(N_CORES):
        xt_i = xg[GPC * i:GPC * (i + 1)].reshape(TOK, D).T
        # block-major layout: [NBLK, DC, 128, BLK] rows contiguous
        xt_i = np.ascontiguousarray(
            xt_i.reshape(DC, 128, NBLK, BLK).transpose(2, 0, 1, 3)
            .reshape(NBLK * DC * 128, BLK)).astype(np.float16)
        in_maps.append({
            "xt": xt_i, "wm": wm, "wn": wn, "bq": bq_eff, "mask": mask,
        })
    res = run_bass_kernel_spmd(nc, in_maps, list(range(N_CORES)), trace=trace)

    outs = np.empty((NGROUP, NB, D), dtype=np.float32)
    for i in range(N_CORES):
        outs[GPC * i:GPC * (i + 1)] = (
            res.results[i]["ot"].astype(np.float32).reshape(GPC, NB, D))
    out = (outs.reshape(B, BL, NB, D).transpose(0, 2, 1, 3)
           .reshape(B, S, D)) + bo_eff
    return out, res


def kernel(x, Wqkv, bqkv, Wout, bout):
    out, _ = run(x, Wqkv, bqkv, Wout, bout, trace=False)
    return out
